# revision 11
# baseline (speedup 1.0000x reference)
"""AttentionBlock kernel for 8 Trainium2 NeuronCores.

Data-parallel over batch: 16 batch elements / 8 cores = 2 per core, full
weights replicated on every core.

Per-core, per-batch pipeline (all matmuls fp32r, full-rate at N>=512):
  1. GroupNorm: per-channel bn_stats, regrouped to per-group records via
     small SBUF->SBUF DMAs, bn_aggr, rstd = exp(-0.5*ln(var+eps)) so the
     whole kernel uses a single ACT table set (natural_log_exp).
  2. Q,K = W_qk @ xn in [c, n] layout; V computed directly transposed
     (vT[m, c] = xn^T @ W_v^T) with a ones column appended per head.
  3. Per head: S^T[m, n] = K^T Q (m on partitions); E = exp(scale*S^T) on
     ACT straight out of PSUM (no max subtraction: logits are ~N(0, 1/9));
     AV accumulates out[c, n] over m-tiles, with the softmax denominator
     d[n] riding the ones column as out row 64.  1/d broadcast across
     partitions on GPSIMD, normalize on DVE.
  4. out = W_p @ attn + b_p + x (residual), DMA back.
"""

import numpy as np

import concourse.bass as bass
import concourse.bacc as bacc
import concourse.tile as tile
from concourse import mybir
from concourse.bass_utils import run_bass_kernel_spmd

F32 = mybir.dt.float32
F32R = mybir.dt.float32r
AF = mybir.ActivationFunctionType
ALU = mybir.AluOpType

B, C, HH, WW = 16, 512, 32, 32
N = HH * WW            # 1024 spatial positions
NH = 8                 # heads
HD = C // NH           # 64 head dim
NG = 32                # groups
EPS = 1e-5
NCORES = 8
BPC = B // NCORES      # batch elements per core
SCALE = HD ** -0.5


def _bcast(sl: bass.AP, repeat: int) -> bass.AP:
    """Insert a [0, repeat] broadcast dim right after the partition dim."""
    return bass.AP(tensor=sl.tensor, offset=sl.offset,
                   ap=[list(sl.ap[0]), [0, repeat]] + [list(a) for a in sl.ap[1:]])


def _emit(nc, tc, x_d, gamma_d, beta_d, qkv_wt_d, qkv_b_d, proj_wt_d,
          proj_b_d, ones_d, out_d, ctx, dbg=None):
    dbg = dbg or {}
    const = ctx.enter_context(tc.tile_pool(name="const", bufs=1))
    sb = ctx.enter_context(tc.tile_pool(name="sb", bufs=1))
    ps = ctx.enter_context(tc.tile_pool(name="ps", bufs=1, space="PSUM"))

    # ---- constants / weights (loaded once) ----
    wq = []
    for ct in range(4):
        w_t = const.tile([128, 3 * C], F32R, tag=f"wq{ct}")
        nc.sync.dma_start(out=w_t, in_=qkv_wt_d[128 * ct:128 * (ct + 1), :])
        wq.append(w_t)
    wp = []
    for ct in range(4):
        p_t = const.tile([128, C], F32R, tag=f"wp{ct}")
        nc.sync.dma_start(out=p_t, in_=proj_wt_d[128 * ct:128 * (ct + 1), :])
        wp.append(p_t)

    gamma_t = const.tile([128, 4], F32)
    nc.sync.dma_start(out=gamma_t, in_=gamma_d[:].rearrange("(t p) -> p t", p=128))
    beta_t = const.tile([128, 4], F32)
    nc.sync.dma_start(out=beta_t, in_=beta_d[:].rearrange("(t p) -> p t", p=128))
    qkvb_t = const.tile([128, 12], F32)
    nc.sync.dma_start(out=qkvb_t, in_=qkv_b_d[:].rearrange("(t p) -> p t", p=128))
    projb_t = const.tile([128, 4], F32)
    nc.sync.dma_start(out=projb_t, in_=proj_b_d[:].rearrange("(t p) -> p t", p=128))
    # v-bias broadcast across partitions: bias_vb[p, c] = qkv_b[1024 + c]
    bias_vb = const.tile([128, C], F32)
    vb_sl = qkv_b_d[2 * C:3 * C]
    nc.sync.dma_start(out=bias_vb, in_=bass.AP(
        tensor=vb_sl.tensor, offset=vb_sl.offset,
        ap=[[0, 128]] + [list(a) for a in vb_sl.ap]))
    eps_t = const.tile([128, 1], F32)
    nc.vector.memset(eps_t, EPS)
    ones_c = const.tile([128, HD], F32R)
    nc.sync.dma_start(out=ones_c, in_=bass.AP(
        tensor=ones_d.tensor, offset=ones_d.offset,
        ap=[[0, 128]] + [list(a) for a in ones_d.ap]))

    for b in range(BPC):
        # ---------------- Phase A: load x + GroupNorm ----------------
        x_t = []
        for ct in range(4):
            xt = sb.tile([128, N], F32, tag="x", bufs=5)
            nc.sync.dma_start(out=xt, in_=x_d[b, 128 * ct:128 * (ct + 1), :])
            x_t.append(xt)

        stats = sb.tile([128, 4, 2, 6], F32, tag="stats", bufs=2)
        for ct in range(4):
            nc.vector.bn_stats(out=stats[:, ct, 0, :], in_=x_t[ct][:, 0:512])
            nc.vector.bn_stats(out=stats[:, ct, 1, :], in_=x_t[ct][:, 512:1024])
        # regroup: agg[g, rec, 6] holds the 32 records (16 ch x 2 halves)
        # of group g; channel c -> group c//16.
        agg = sb.tile([32, 32, 6], F32, tag="agg", bufs=2)
        for ct in range(4):
            nc.sync.dma_start(out=agg[8 * ct:8 * ct + 8, :, :],
                              in_=stats[:, ct, :, :])
        mv = sb.tile([32, 2], F32, tag="mv", bufs=2)
        nc.vector.bn_aggr(out=mv, in_=agg)
        # mvr: col0 = mean, col1 = rstd = exp(-0.5 * ln(var + eps))
        ln_t = sb.tile([32, 1], F32, tag="ln", bufs=2)
        nc.scalar.activation(out=ln_t, in_=mv[:, 1:2], func=AF.Ln,
                             bias=eps_t[0:32, :], scale=1.0)
        mvr = sb.tile([32, 2], F32, tag="mvr", bufs=2)
        nc.vector.tensor_copy(mvr[:, 0:1], mv[:, 0:1])
        nc.scalar.activation(out=mvr[:, 1:2], in_=ln_t, func=AF.Exp,
                             bias=0.0, scale=-0.5)
        # broadcast group stats to channels: mr_ch[p, ct, 0:2]
        mr_ch = sb.tile([128, 4, 2], F32, tag="mr_ch", bufs=2)
        for ct in range(4):
            nc.sync.dma_start(out=mr_ch[:, ct, :],
                              in_=_bcast(mvr[8 * ct:8 * ct + 8, :], 16))
        scale_t = sb.tile([128, 4], F32, tag="scale_t", bufs=2)
        nc.vector.tensor_tensor(out=scale_t, in0=mr_ch[:, :, 1],
                                in1=gamma_t, op=ALU.mult)
        off_t = sb.tile([128, 4], F32, tag="off_t", bufs=2)
        nc.vector.tensor_tensor(out=off_t, in0=mr_ch[:, :, 0],
                                in1=scale_t, op=ALU.mult)
        nc.vector.tensor_tensor(out=off_t, in0=beta_t, in1=off_t,
                                op=ALU.subtract)
        xn = []
        for ct in range(4):
            xnt = sb.tile([128, N], F32R, tag="xn", bufs=5)
            nc.vector.tensor_scalar(
                out=xnt, in0=x_t[ct], scalar1=scale_t[:, ct:ct + 1],
                scalar2=off_t[:, ct:ct + 1], op0=ALU.mult, op1=ALU.add)
            xn.append(xnt)
            if b == 0 and "xn" in dbg:
                nc.sync.dma_start(out=dbg["xn"][ct], in_=xnt.bitcast(F32))

        # ---------------- Phase B: Q, K = W_qk @ xn ----------------
        qk = []
        for to in range(8):
            pqk = ps.tile([128, N], F32, tag="s", bufs=2)
            for ct in range(4):
                for ch in range(2):
                    nc.tensor.matmul(
                        pqk[:, 512 * ch:512 * (ch + 1)],
                        wq[ct][:, 128 * to:128 * (to + 1)],
                        xn[ct][:, 512 * ch:512 * (ch + 1)],
                        start=(ct == 0), stop=(ct == 3))
            qkt = sb.tile([128, N], F32R, tag="qk", bufs=8)
            nc.vector.tensor_scalar(out=qkt, in0=pqk,
                                    scalar1=qkvb_t[:, to:to + 1], scalar2=None,
                                    op0=ALU.add)
            qk.append(qkt)
            if b == 0 and "qk" in dbg and to in (0, 4):
                nc.sync.dma_start(out=dbg["qk"][to // 4], in_=qkt.bitcast(F32))

        # ---------------- Phase B2: vT[m, c] (+ ones col per head) ----
        vt = sb.tile([128, 8, NH, HD + 1], F32R, tag="vt", bufs=1)
        ones_sl = vt[:, 0, 0, HD:HD + 1]
        nc.sync.dma_start(
            out=bass.AP(tensor=ones_sl.tensor, offset=ones_sl.offset,
                        ap=[list(ones_sl.ap[0]), [HD + 1, 64]]),
            in_=ones_c)
        for mt in range(8):
            pv = ps.tile([128, N], F32, tag="s", bufs=2)
            for ct in range(4):
                nc.tensor.matmul(
                    pv[:, 0:512],
                    xn[ct][:, 128 * mt:128 * (mt + 1)],
                    wq[ct][:, 2 * C:3 * C],
                    start=(ct == 0), stop=(ct == 3))
            nc.vector.tensor_tensor(out=vt[:, mt, :, 0:HD], in0=pv[:, 0:512],
                                    in1=bias_vb, op=ALU.add)
        if b == 0 and "vt" in dbg:
            nc.sync.dma_start(out=dbg["vt"][:], in_=vt.bitcast(F32))

        # ---------------- Phase C: attention per head ----------------
        attnout = []
        for ct in range(4):
            at = sb.tile([128, N], F32R, tag="attn", bufs=4)
            attnout.append(at)
        for h in range(NH):
            pbase = 64 * (h % 2)
            q_h = qk[h // 2][pbase:pbase + 64, :]
            k_h = qk[4 + h // 2][pbase:pbase + 64, :]
            av = ps.tile([128, N], F32, tag="av", bufs=2)
            for mt in range(8):
                s_ps = ps.tile([128, N], F32, tag="s", bufs=2)
                for ch in range(2):
                    nc.tensor.matmul(
                        s_ps[:, 512 * ch:512 * (ch + 1)],
                        k_h[:, 128 * mt:128 * (mt + 1)],
                        q_h[:, 512 * ch:512 * (ch + 1)],
                        start=True, stop=True)
                e_t = sb.tile([128, N], F32R, tag="e", bufs=3)
                nc.scalar.activation(out=e_t, in_=s_ps, func=AF.Exp,
                                     bias=0.0, scale=SCALE)
                if b == 0 and h == 0 and mt == 0 and "e" in dbg:
                    nc.sync.dma_start(out=dbg["e"][:], in_=e_t.bitcast(F32))
                for ch in range(2):
                    nc.tensor.matmul(
                        av[0:HD + 1, 512 * ch:512 * (ch + 1)],
                        vt[:, mt, h, :],
                        e_t[:, 512 * ch:512 * (ch + 1)],
                        start=(mt == 0), stop=(mt == 7))
            if b == 0 and h == 0 and "av" in dbg:
                av_dbg = sb.tile([128, N], F32, tag="av_dbg", bufs=1)
                nc.vector.tensor_copy(av_dbg, av)
                nc.sync.dma_start(out=dbg["av"][:], in_=av_dbg)
            rd = sb.tile([128, N], F32, tag="rd", bufs=1)
            nc.vector.reciprocal(out=rd[HD:HD + 1, :], in_=av[HD:HD + 1, :])
            bc = sb.tile([128, N], F32, tag="bc", bufs=2)
            nc.sync.dma_start(out=bc[0:HD, :],
                              in_=_bcast(rd[HD:HD + 1, :], HD))
            if b == 0 and h == 0 and "bc" in dbg:
                nc.sync.dma_start(out=dbg["bc"][:], in_=bc)
            dst = attnout[h // 2]
            if h % 2 == 0:
                nc.vector.tensor_tensor(out=dst[0:64, :], in0=av[0:64, :],
                                        in1=bc[0:64, :], op=ALU.mult)
            else:
                stg = sb.tile([128, N], F32R, tag="stg", bufs=1)
                nc.vector.tensor_tensor(out=stg[0:64, :], in0=av[0:64, :],
                                        in1=bc[0:64, :], op=ALU.mult)
                nc.sync.dma_start(out=dst[64:128, :], in_=stg[0:64, :])

        if b == 0 and "attn" in dbg:
            for ct in range(4):
                nc.sync.dma_start(out=dbg["attn"][ct],
                                  in_=attnout[ct].bitcast(F32))

        # ---------------- Phase D: proj + bias + residual ----------------
        for to in range(4):
            pp = ps.tile([128, N], F32, tag="s", bufs=2)
            for ct in range(4):
                for ch in range(2):
                    nc.tensor.matmul(
                        pp[:, 512 * ch:512 * (ch + 1)],
                        wp[ct][:, 128 * to:128 * (to + 1)],
                        attnout[ct][:, 512 * ch:512 * (ch + 1)],
                        start=(ct == 0), stop=(ct == 3))
            tmp = sb.tile([128, N], F32, tag="tmp", bufs=1)
            nc.vector.tensor_scalar(out=tmp, in0=pp,
                                    scalar1=projb_t[:, to:to + 1], scalar2=None,
                                    op0=ALU.add)
            ot = sb.tile([128, N], F32, tag="ot", bufs=2)
            nc.vector.tensor_tensor(out=ot, in0=tmp, in1=x_t[to], op=ALU.add)
            nc.sync.dma_start(out=out_d[b, 128 * to:128 * (to + 1), :], in_=ot)


_CACHE = {}
DEBUG = False


def build():
    if "nc" in _CACHE:
        return _CACHE["nc"]
    from contextlib import ExitStack
    nc = bacc.Bacc(trn_type="TRN2", dynamic_dma_scratch_size=512)
    x_d = nc.declare_dram_parameter("x", [BPC, C, N], F32, isOutput=False)
    gamma_d = nc.declare_dram_parameter("gamma", [C], F32, isOutput=False)
    beta_d = nc.declare_dram_parameter("beta", [C], F32, isOutput=False)
    qkv_wt_d = nc.declare_dram_parameter("qkv_wt", [C, 3 * C], F32R,
                                         isOutput=False)
    qkv_b_d = nc.declare_dram_parameter("qkv_b", [3 * C], F32, isOutput=False)
    proj_wt_d = nc.declare_dram_parameter("proj_wt", [C, C], F32R,
                                          isOutput=False)
    proj_b_d = nc.declare_dram_parameter("proj_b", [C], F32, isOutput=False)
    ones_d = nc.declare_dram_parameter("ones", [HD], F32R, isOutput=False)
    out_d = nc.declare_dram_parameter("out", [BPC, C, N], F32, isOutput=True)
    dbg = {}
    if DEBUG:
        dbg["xn"] = nc.declare_dram_parameter("dbg_xn", [4, 128, N], F32, isOutput=True)
        dbg["qk"] = nc.declare_dram_parameter("dbg_qk", [2, 128, N], F32, isOutput=True)
        dbg["vt"] = nc.declare_dram_parameter("dbg_vt", [128, 8, NH, HD + 1], F32, isOutput=True)
        dbg["e"] = nc.declare_dram_parameter("dbg_e", [128, N], F32, isOutput=True)
        dbg["av"] = nc.declare_dram_parameter("dbg_av", [128, N], F32, isOutput=True)
        dbg["bc"] = nc.declare_dram_parameter("dbg_bc", [128, N], F32, isOutput=True)
        dbg["attn"] = nc.declare_dram_parameter("dbg_attn", [4, 128, N], F32, isOutput=True)
    with tile.TileContext(nc) as tc:
        with ExitStack() as ctx:
            _emit(nc, tc, x_d[:], gamma_d[:], beta_d[:], qkv_wt_d[:],
                  qkv_b_d[:], proj_wt_d[:], proj_b_d[:], ones_d[:], out_d[:],
                  ctx, dbg)
    nc.compile()
    _CACHE["nc"] = nc
    return nc


def run(x, norm_gamma, norm_beta, qkv_w, qkv_b, proj_w, proj_b, **kw):
    nc = build()
    xr = np.ascontiguousarray(
        np.asarray(x, dtype=np.float32).reshape(B, C, N))
    qkv_wt = np.ascontiguousarray(np.asarray(qkv_w, dtype=np.float32).T)
    proj_wt = np.ascontiguousarray(np.asarray(proj_w, dtype=np.float32).T)
    shared = {
        "gamma": np.ascontiguousarray(np.asarray(norm_gamma, np.float32)),
        "beta": np.ascontiguousarray(np.asarray(norm_beta, np.float32)),
        "qkv_wt": qkv_wt,
        "qkv_b": np.ascontiguousarray(np.asarray(qkv_b, np.float32)),
        "proj_wt": proj_wt,
        "proj_b": np.ascontiguousarray(np.asarray(proj_b, np.float32)),
        "ones": np.ones(HD, dtype=np.float32),
    }
    in_maps = []
    for c in range(NCORES):
        m = dict(shared)
        m["x"] = np.ascontiguousarray(xr[BPC * c:BPC * (c + 1)])
        in_maps.append(m)
    res = run_bass_kernel_spmd(nc, in_maps, core_ids=list(range(NCORES)), **kw)
    out = np.empty((B, C, N), dtype=np.float32)
    for c in range(NCORES):
        out[BPC * c:BPC * (c + 1)] = res.results[c]["out"]
    return out.reshape(B, C, HH, WW), res


def kernel(**inputs) -> np.ndarray:
    out, _ = run(**inputs)
    return out
